# revision 36
# baseline (speedup 1.0000x reference)
"""ArcFace head forward on 8 Trainium2 NeuronCores (Bass, raw blocks) — v2.

Math (per batch row b, class c in {0,1}):
    feat_n = feat / max(||feat||, 1e-12)
    W_n    = W / max(||W_row||, 1e-12)
    cos    = feat_n . W_n[c]                  (|cos| << 1 for this data,
                                               so clips never bind)
    cos_m  = cos*cos(0.5) - sqrt(1-cos^2)*sin(0.5)
    out    = 64 * (cos if c != label[b] else cos_m)

Distribution: pure data parallel, feat/label sharded along batch over 8
cores, W normalized on host and replicated; forward only, no collectives.

Design (all trace-driven):
  - Transposes are plain bf16 128x128 PE transposes (8 per pair): bf16
    LDWEIGHTS gets fast-weight-load and hides under the previous matmul,
    so sustained cost is ~56-110ns per transpose vs ~161-215ns for the
    f32-packed alternative.  4 transpose PSUM banks (pair u -> tpb[u%4])
    + 4 dot-strip banks; dots run LAG=2 pairs behind transposes.
  - Preamble is ONE [128, 208] f32 DMA (wrepT | lab | bf16-identity
    packed in f32) issued FIRST on the SWDGE queue: the HWDGE path takes
    ~3us to first byte at kernel start and gated PE's first transposes.
  - ACT/DVE start on feat data immediately (block-0 sumsqs gate on the
    piece sems); preamble-dependent prep (W cast, one-hot/A_k) is
    deferred past it.
  - Feat pieces: b0 in 5, b1-b14 whole, b15 in 5 for drain overlap.
  - Epilogue chunks are UNEVEN: [5,5,4,2] blocks: chunks 0-2 complete
    inside the stream even when a slow SDMA engine compresses the last
    blocks' arrivals; only the final 2-block chain sits in the drain,
    with block-15 sumsqs split ACT(j0,j1,j4)/DVE(j2,j3,j5,j6,j7).
  - Blend uses the 3-op form out = cc*A_k - sins*oh_k with
    A_k = 1 + (cos m - 1)*oh_k precomputed once.
  - Output writes are deferred into the drain (mid-stream HBM-write
    receipts occupy an SDMA engine slot and steal feat bandwidth); the
    tiny final write is issued from the scalar engine.
  - Explicit pedot gate before each chunk harvest (DVE read of a dstr
    bank must not race PE dot writes: PSUM bank collision is fatal).

Hardware landmines kept from v1:
  - Strided (stride>=2) PSUM reads corrupt beyond 64 B -> harvest each
    dot strip with a contiguous tensor_copy to SBUF, split classes there.
  - DVE InstReciprocal pipeline drain outlives retirement -> reciprocal
    kept >=32 lanes wide (windows that end at the chunk boundary) and
    consumers gate on its completion semaphore (vrcp).
  - matmul start=True clears has_written for the whole PSUM bank -> a
    sub-tile's 4 accumulating dots are consecutive; strips already final
    only lose their (irrelevant) has_written bits.
Row mapping: batch row b = blk*1024 + p*8 + j lives on partition p,
accumulator column t = blk*8 + j.  Out DRAM layout t-major interleaved
out[p, t*2+c] so staged writes are contiguous per partition.
"""

import sys
from contextlib import ExitStack

import numpy as np

for _p in ("/opt/trn_rl_repo",):
    if _p not in sys.path:
        sys.path.insert(0, _p)

import concourse.bass as bass
import concourse.mybir as mybir
from concourse.bass_utils import run_bass_kernel_spmd

B, D, C = 131072, 512, 2
NCORES = 8
BS = B // NCORES          # 16384 rows per core
SUB = 512                 # bf16 columns per sub-tile (one 128-row group)
SPB = 8                   # sub-tiles per block
BLK_COLS = SUB * SPB      # 4096 bf16 cols (1024 batch rows)
NBLK = BS // (128 * SPB)  # 16
T = BS // 128             # 128 accumulator columns
PAIRS = NBLK * SPB // 2   # 64 pairs (2 sub-tiles each)
PPB = SPB // 2            # pairs per block (4)
LAG = 2                   # dots run LAG pairs behind transposes
CHUNK_BLKS = [5, 5, 4, 2]
NCH = len(CHUNK_BLKS)
CHUNK_B0 = [0, 5, 10, 14]            # first block of each chunk
CHUNK_T0 = [0, 40, 80, 112]          # first t-col of each chunk
CHUNK_TC = [40, 40, 32, 16]          # t-cols per chunk
SQ_ACT = (0, 1)                      # sub-tiles whose sumsq runs on ACT

S_SCALE = 64.0
MARGIN = 0.5
COS_M = float(np.cos(MARGIN))
SIN_M = float(np.sin(MARGIN))
S2SIN2 = float((S_SCALE * np.sin(MARGIN)) ** 2)   # (64 sin m)^2
SIN_M2 = float(np.sin(MARGIN) ** 2)
INV_S2 = float(1.0 / (S_SCALE * S_SCALE))         # 1/4096

F32 = mybir.dt.float32
BF16 = mybir.dt.bfloat16
DEBUG = False  # set before first build_nc() to add intermediate dumps

# feat pieces: (block, col0, col1) in bf16 block-columns; one DMA + sem
# each.  Matches the v1 baseline's emission geometry exactly (which
# measured a balanced per-SDMA-engine load).
PIECES = (
    [(0, 0, 512), (0, 512, 1024), (0, 1024, 2048), (0, 2048, 3072),
     (0, 3072, 4096)]
    + [(i, 0, 4096) for i in range(1, NBLK - 1)]
    + [(15, 0, 1024), (15, 1024, 2048), (15, 2048, 3072),
       (15, 3072, 3584), (15, 3584, 4096)]
)


def _piece_idx(blk, col):
    """Index of the piece containing (blk, col)."""
    for k, (b, c0, c1) in enumerate(PIECES):
        if b == blk and c0 <= col < c1:
            return k
    raise ValueError((blk, col))


def build_nc():
    nc = bass.Bass()
    AF = mybir.ActivationFunctionType
    OP = mybir.AluOpType

    feat = nc.declare_dram_parameter("feat", [BS, D], F32, isOutput=False)
    # pre[:, 0:9]    = wrepT (w_bfT source: [p, q*2+c] = Wn[c, q*128+p]) + s2sin2
    # pre[:, 9:137]  = lab_t   [p, t] = label[blk*1024 + p*8 + j], t = blk*8+j
    # pre[:, 137:201]= bf16 identity packed as f32 pairs
    pre_d = nc.declare_dram_parameter("pre", [128, 208], F32, isOutput=False)
    out = nc.declare_dram_parameter("out", [128, T * C], F32, isOutput=True)
    if DEBUG:
        dbg_ss = nc.declare_dram_parameter("dbg_ss", [128, T], F32, isOutput=True)
        dbg_norm = nc.declare_dram_parameter("dbg_norm", [128, T], F32, isOutput=True)
        dbg_cc0 = nc.declare_dram_parameter("dbg_cc0", [128, T], F32, isOutput=True)
        dbg_ft = nc.declare_dram_parameter("dbg_ft", [128, 512], F32, isOutput=True)
        dbg_fb = nc.declare_dram_parameter("dbg_fb", [128, 256], F32, isOutput=True)
        dbg_w = nc.declare_dram_parameter("dbg_w", [128, 4], F32, isOutput=True)
        dbg_sqs0 = nc.declare_dram_parameter("dbg_sqs0", [128, T], F32, isOutput=True)
        dbg_sins0 = nc.declare_dram_parameter("dbg_sins0", [128, T], F32, isOutput=True)
        dbg_ostage = nc.declare_dram_parameter("dbg_ostage", [128, T * C], F32, isOutput=True)

    feat_v = feat[:].rearrange("(blk p j) d -> blk p (j d)", p=128, j=SPB)

    with ExitStack() as ctx:
        def sb(name, shape, dt):
            return ctx.enter_context(nc.sbuf_tensor(name, shape, dt))

        def psum(name, shape, dt):
            return ctx.enter_context(nc.psum_tensor(name, shape, dt))

        def sem(name):
            return ctx.enter_context(nc.semaphore(name))

        pre_sb = sb("pre_sb", [128, 208], F32)
        w_bf = sb("w_bf", [128, 8], BF16)      # [p, q*2+c] = Wn[c, q*128+p]
        ss = sb("ss", [128, T], F32)
        fbs = [sb(f"fb{k}", [128, BLK_COLS], BF16) for k in range(NBLK)]
        ftT = [sb(f"ftT{k}", [128, 2 * SUB], BF16) for k in range(3)]
        scr_bf = sb("scr_bf", [128, SUB], BF16)
        tt_scr = sb("tt_scr", [128, SUB], BF16)
        norm = sb("norm", [128, T], F32)
        invs = sb("invs", [128, T], F32)
        cc = [sb(f"cc{k}", [128, T], F32) for k in range(C)]
        sqs = [sb(f"sqs{k}", [128, T], F32) for k in range(C)]
        sins = [sb(f"sins{k}", [128, T], F32) for k in range(C)]
        tmp1 = sb("tmp1", [128, T], F32)
        tmp2 = sb("tmp2", [128, T], F32)
        aa = [sb(f"aa{k}", [128, T], F32) for k in range(C)]  # 1+(cos m-1)*oh_k
        # persistent dot harvest, t-major [p, (t c)] — persists across chunks
        # so chunk 3's 32-wide window ops can harmlessly recompute chunk 2's
        # tail (all DVE/ACT epilogue ops must be >=32 lanes wide: narrower
        # DVE ops read stale data from the immediately preceding op)
        dots_sb = sb("dots_sb", [128, T * C], F32)
        oh0 = sb("oh0", [128, T], F32)
        out_stage = sb("out_stage", [128, T * C], F32)
        out_stage_v = out_stage[:].rearrange("p (t c) -> p c t", c=C)

        lab_t = pre_sb[:, 9:137]
        idb = pre_sb[:, 137:201].bitcast(BF16)   # [128, 128] bf16 identity

        tpb = [psum(f"tpb{k}", [128, 2 * SUB], BF16) for k in range(4)]
        dstr = [psum(f"dstr{c}", [128, CHUNK_BLKS[c] * SPB * C], F32)
                for c in range(NCH)]

        pre = sem("pre")
        lds = [sem(f"ld{k}") for k in range(len(PIECES))]
        petr = sem("petr")     # PE transposes done for pair u -> u+1
        actcopy = sem("actcopy")  # ACT copy of pair u done -> u+1
        pedot = sem("pedot")   # PE dots done for pair -> +1
        ssd = sem("ssd")       # all sumsq cols of chunk c written -> c+1
        sepn = sem("sepn")     # ACT norm for chunk c done -> c+1
        seps = sem("seps")     # ACT sinS pair for chunk c done -> c+1
        vepic = sem("vepic")   # DVE C/sq' phase for chunk c done -> c+1
        vepio = sem("vepio")   # DVE out_stage chunk c written -> c+1
        vrcp = sem("vrcp")     # reciprocal retired
        outd = sem("outd")

        # chunk -> pair count needed for all its dots: pairs [0, 4*(b0+nb))
        chunk_pairs = [4 * (CHUNK_B0[c] + CHUNK_BLKS[c]) for c in range(NCH)]

        with nc.Block() as block:

            @block.sync
            def _(sync):
                # Output writes are DEFERRED into the drain: a mid-stream
                # HBM write's completion receipt (~2-3us) occupies an SDMA
                # engine slot (observed ~8-9us of engine-0 busy), stealing
                # feat-stream bandwidth from that engine.  After the last
                # feat piece, HBM is idle and the write overlaps the drain
                # compute on the other engines.  (The preamble load is
                # issued by gpsimd/SWDGE: the HWDGE path takes ~3us to
                # first byte at kernel start and was gating PE's first
                # transposes on the identity matrix.)
                sync.wait_ge(lds[len(PIECES) - 1], 16)
                sync.wait_ge(vepio, 2)
                sync.dma_start(
                    out=out[:, 0:160], in_=out_stage[:, 0:160]
                ).then_inc(outd, 16)
                sync.wait_ge(vepio, 3)
                sync.dma_start(
                    out=out[:, 160:224], in_=out_stage[:, 160:224]
                ).then_inc(outd, 16)
                # third (final, tiny) write is issued by the scalar engine
                sync.wait_ge(outd, 48)
                if DEBUG:  # noqa: outd counts below assume the 2 writes above
                    # all compute is done once the last out chunk left
                    sync.dma_start(out=dbg_ss[:], in_=ss[:]).then_inc(outd, 16)
                    sync.dma_start(out=dbg_norm[:], in_=norm[:]).then_inc(outd, 16)
                    sync.dma_start(out=dbg_cc0[:], in_=cc[0][:]).then_inc(outd, 16)
                    # ftT[0] holds pair 63 (63 % 3 == 0): block 15 cols 3072:4096
                    sync.dma_start(
                        out=dbg_ft[:], in_=ftT[0][:].bitcast(F32)
                    ).then_inc(outd, 16)
                    sync.dma_start(
                        out=dbg_fb[:], in_=fbs[0][:, 0:512].bitcast(F32)
                    ).then_inc(outd, 16)
                    sync.dma_start(
                        out=dbg_w[:], in_=w_bf[:].bitcast(F32)
                    ).then_inc(outd, 16)
                    sync.dma_start(out=dbg_sqs0[:], in_=sqs[0][:]).then_inc(outd, 16)
                    sync.dma_start(out=dbg_sins0[:], in_=sins[0][:]).then_inc(outd, 16)
                    sync.dma_start(out=dbg_ostage[:], in_=out_stage[:]).then_inc(outd, 16)
                    sync.wait_ge(outd, 48 + 9 * 16)

            @block.gpsimd
            def _(gpsimd):
                # preamble rides the SWDGE queue first: it is tiny (104 KB)
                # and everything (identity, W, labels) gates on it
                gpsimd.dma_start(out=pre_sb[:], in_=pre_d[:]).then_inc(pre, 16)
                for k, (b, c0, c1) in enumerate(PIECES):
                    gpsimd.dma_start(
                        out=fbs[b][:, c0:c1], in_=feat_v[b][:, c0:c1]
                    ).then_inc(lds[k], 16)

            @block.tensor
            def _(tensor):
                tensor.wait_ge(pre, 16)  # idb ready
                for u in range(PAIRS + LAG):
                    if u < PAIRS:
                        i = u // PPB
                        # gate on the piece(s) covering this pair's columns
                        pair_c0 = (u % PPB) * 2 * SUB
                        waited = set()
                        for q in range(8):
                            k = _piece_idx(i, pair_c0 + q * 128)
                            if k not in waited:
                                waited.add(k)
                                tensor.wait_ge(lds[k], 16)
                            ins = tensor.transpose(
                                tpb[u % 4][:, q * 128:(q + 1) * 128],
                                fbs[i][:, pair_c0 + q * 128:
                                       pair_c0 + (q + 1) * 128],
                                idb,
                            )
                        ins.then_inc(petr, 1)
                    if u >= LAG:
                        ud = u - LAG
                        tensor.wait_ge(actcopy, ud + 1)
                        i_d = ud // PPB
                        ch = next(c for c in range(NCH)
                                  if CHUNK_B0[c] <= i_d
                                  < CHUNK_B0[c] + CHUNK_BLKS[c])
                        lb = i_d - CHUNK_B0[ch]
                        for s in range(2):
                            j_d = (ud % PPB) * 2 + s
                            off = (lb * SPB + j_d) * C
                            dsl = dstr[ch][:, off:off + C]
                            for q in range(4):
                                ins = tensor.matmul(
                                    dsl,
                                    ftT[ud % 3][:, (s * 4 + q) * 128:
                                                (s * 4 + q + 1) * 128],
                                    w_bf[:, q * 2:(q + 1) * 2],
                                    start=(q == 0), stop=(q == 3),
                                )
                        ins.then_inc(pedot, 1)

            @block.scalar
            def _(scalar):
                # start on feat data immediately (lds gates); the preamble
                # sem's ~2.5us completion receipt would otherwise idle ACT
                # while block 0 is already in SBUF
                scalar.wait_ge(lds[0], 16)
                scalar.activation(
                    out=scr_bf[:], in_=fbs[0][:, 0:SUB], func=AF.Square,
                    accum_out=ss[:, 0:1],
                )
                scalar.wait_ge(lds[1], 16)
                scalar.activation(
                    out=scr_bf[:], in_=fbs[0][:, SUB:2 * SUB], func=AF.Square,
                    accum_out=ss[:, 1:2],
                )
                scalar.wait_ge(pre, 16)
                # w_bf = wrepT cast f32 -> bf16 (already in [p, q*2+c] layout)
                scalar.activation(out=w_bf[:], in_=pre_sb[:, 0:8], func=AF.Copy)

                def wsl(c):
                    # >=32-wide op window ending at the chunk end (chunk 3 is
                    # 8 cols; its window recomputes chunk 2's tail, which is
                    # idempotent: same inputs -> same values)
                    t1 = CHUNK_T0[c] + CHUNK_TC[c]
                    return slice(min(CHUNK_T0[c], t1 - 32), t1)

                def chunk_norm(c):
                    sl = wsl(c)
                    scalar.wait_ge(ssd, c + 1)
                    scalar.activation(
                        out=norm[:, sl], in_=ss[:, sl], func=AF.Sqrt,
                        scale=INV_S2,
                    ).then_inc(sepn, 1)

                def chunk_sin(c):
                    sl = wsl(c)
                    scalar.wait_ge(vepic, c + 1)
                    for k in range(C):
                        ins = scalar.activation(
                            out=sins[k][:, sl], in_=sqs[k][:, sl],
                            func=AF.Sqrt, bias=pre_sb[:, 8:9], scale=-1.0,
                        )
                    ins.then_inc(seps, 1)

                def pcopy(u):
                    scalar.wait_ge(petr, u + 1)
                    scalar.activation(
                        out=ftT[u % 3][:].bitcast(F32),
                        in_=tpb[u % 4][:].bitcast(F32), func=AF.Copy,
                    ).then_inc(actcopy, 1)

                def sq(i, j):
                    scalar.activation(
                        out=scr_bf[:],
                        in_=fbs[i][:, j * SUB:(j + 1) * SUB],
                        func=AF.Square,
                        accum_out=ss[:, i * SPB + j:i * SPB + j + 1],
                    )

                # per-block ACT schedule; chunk milestones interleave mid-
                # block so they never sit in front of a petr-gated pcopy
                # whose data lands later than the milestone's own gate.
                for i in range(NBLK):
                    u0 = i * PPB
                    pcopy(u0 + 0)
                    if i > 0:  # block 0's j0/j1 ran before the W cast above
                        sq(i, SQ_ACT[0])
                    pcopy(u0 + 1)
                    if i > 0:
                        sq(i, SQ_ACT[1])
                    if i == NBLK - 1:
                        # block-15 j4 as early as its piece allows: it is
                        # the last ACT input of norm(3)'s window, and queued
                        # any later it serializes the whole drain epilogue
                        scalar.wait_ge(lds[_piece_idx(NBLK - 1, 4 * SUB)], 16)
                        sq(NBLK - 1, 4)
                        # sin(2)'s gate (vepic3) fired back at block 14;
                        # running it here keeps the two drain pair-copies
                        # back-to-back at the front of the ACT queue
                        chunk_sin(2)
                    if i == 6:
                        chunk_norm(0)
                    elif i == 7:
                        chunk_sin(0)
                    elif i == 11:
                        chunk_norm(1)
                    elif i == 12:
                        chunk_sin(1)
                    elif i == 14:
                        chunk_norm(2)
                    pcopy(u0 + 2)
                    pcopy(u0 + 3)
                # norm(3) after both drain pair-copies: with the DVE sumsqs
                # in piece-arrival order its ssd4 gate fires while pcopy(63)
                # is still in flight, so this order starts the dots sooner
                chunk_norm(3)
                chunk_sin(3)
                # final 8KB out write issued from here: the ACT sequencer is
                # already awake, saving the sync engine's wakeup latency
                scalar.wait_ge(vepio, 4)
                # 512B/partition so the SDMA write skips the sub-512B
                # read-modify-write on its receipt path (cols 128:224 are
                # rewritten with identical values)
                scalar.dma_start(
                    out=out[:, 128:256], in_=out_stage[:, 128:256]
                ).then_inc(outd, 16)

            @block.vector
            def _(vector):
                pass  # oh0/aa prep is deferred into the block loop (i == 1)
                # so block-0 sumsqs start on feat data without waiting for
                # the preamble DMA's completion receipt

                def wsl(c):
                    t1 = CHUNK_T0[c] + CHUNK_TC[c]
                    return slice(min(CHUNK_T0[c], t1 - 32), t1)

                def chunk_phase_a(c):
                    sl = wsl(c)
                    t0, t1 = sl.start, sl.stop
                    # reciprocal first (its sepn gate fires before the last
                    # pair's copy+dots chain completes), then the pedot-gated
                    # harvest: the fat FD>=32 copy also covers the recip's
                    # pipeline drain, and vrcp gates the consumers.
                    vector.wait_ge(sepn, c + 1)
                    vector.reciprocal(
                        invs[:, sl], norm[:, sl]
                    ).then_inc(vrcp, 1)
                    vector.wait_ge(pedot, chunk_pairs[c])
                    o0 = CHUNK_T0[c] * C
                    vector.tensor_copy(
                        dots_sb[:, o0:o0 + CHUNK_BLKS[c] * SPB * C],
                        dstr[c][:])
                    vector.wait_ge(vrcp, c + 1)
                    for k in range(C):
                        dk = dots_sb[:, t0 * C:t1 * C].rearrange(
                            "p (b c) -> p c b", c=C)[:, k, :]
                        vector.tensor_tensor(cc[k][:, sl], dk, invs[:, sl],
                                             OP.mult)
                        ins = vector.scalar_tensor_tensor(
                            out=sqs[k][:, sl], in0=cc[k][:, sl],
                            scalar=SIN_M2, in1=cc[k][:, sl],
                            op0=OP.mult, op1=OP.mult,
                        )
                    ins.then_inc(vepic, 1)

                def chunk_phase_b(c):
                    # out = cc + (cos_m*cc - sins - cc)*oh
                    #     = cc*(1 + (cos_m - 1)*oh) - sins*oh = cc*A - sins*oh
                    sl = wsl(c)
                    vector.wait_ge(seps, c + 1)
                    for k in range(C):
                        oh = oh0 if k == 0 else lab_t
                        vector.tensor_tensor(tmp1[:, sl], cc[k][:, sl],
                                             aa[k][:, sl], OP.mult)
                        vector.tensor_tensor(tmp2[:, sl], sins[k][:, sl],
                                             oh[:, sl], OP.mult)
                        ins = vector.tensor_tensor(
                            out_stage_v[:, k, sl], tmp1[:, sl],
                            tmp2[:, sl], OP.subtract)
                    ins.then_inc(vepio, 1)

                def sq_dve(i, j, inc_ssd=False):
                    vector.wait_ge(lds[_piece_idx(i, j * SUB)], 16)
                    sl = slice(j * SUB, (j + 1) * SUB)
                    ins = vector.scalar_tensor_tensor(
                        out=tt_scr[:], in0=fbs[i][:, sl], scalar=1.0,
                        in1=fbs[i][:, sl], op0=OP.mult, op1=OP.mult,
                        accum_out=ss[:, i * SPB + j:i * SPB + j + 1],
                    )
                    # ssd milestone: last DVE sumsq col of a chunk. ACT's
                    # j=0,1 of the same block always complete earlier (ACT
                    # does them at block start), so DVE's j=7 is the gate.
                    if inc_ssd:
                        ins.then_inc(ssd, 1)

                for i in range(NBLK - 1):
                    if i == 1:
                        vector.wait_ge(pre, 16)
                        vector.tensor_scalar(oh0[:], lab_t, -1.0, 1.0,
                                             OP.mult, OP.add)
                        # A_k = 1 + (cos m - 1)*oh_k for the 3-op blend:
                        # A0 = lab*(1-cos m) + cos m ; A1 = lab*(cos m-1) + 1
                        vector.tensor_scalar(aa[0][:], lab_t, 1.0 - COS_M,
                                             COS_M, OP.mult, OP.add)
                        vector.tensor_scalar(aa[1][:], lab_t, COS_M - 1.0,
                                             1.0, OP.mult, OP.add)
                    for j in range(SPB):
                        if j in SQ_ACT:
                            continue
                        sq_dve(i, j, inc_ssd=(j == SPB - 1
                                              and (i + 1) in [5, 10, 14]))
                    if i == 6:
                        chunk_phase_a(0)
                    elif i == 7:
                        chunk_phase_b(0)
                    elif i == 11:
                        chunk_phase_a(1)
                    elif i == 12:
                        chunk_phase_b(1)
                    elif i == 14:
                        chunk_phase_a(2)
                # block 15: interleave chunk-2 epilogue with the piece-gated
                # sumsq so chunks 0-2 never wait on the final DMA pieces.
                # piece-arrival order (j5's piece lands before j6's/j7's);
                # the ssd inc rides the last one, j7, covering all DVE
                # chunk-3 cols (ACT's j0/j1/j4 precede norm(3) in ACT order)
                sq_dve(15, 2)
                sq_dve(15, 3)
                sq_dve(15, 5)
                sq_dve(15, 6)
                sq_dve(15, 7, inc_ssd=True)
                # chunk-3's phase_a runs first so its reciprocal/harvest
                # never wait behind phase_b(2); phase_b(2) still precedes
                # phase_b(3) in program order, keeping the vepio incs (and
                # therefore the staged out-writes) in chunk order.
                chunk_phase_a(3)
                chunk_phase_b(2)
                chunk_phase_b(3)

    return nc


_NC = None


def _get_nc():
    global _NC
    if _NC is None:
        _NC = build_nc()
    return _NC


def _bf16_bits(x):
    u = np.asarray(x, dtype=np.float32).view(np.uint32)
    return ((u + 0x7FFF + ((u >> 16) & 1)) >> 16).astype(np.uint16)


def _make_in_maps(feat, W, label):
    feat = np.ascontiguousarray(np.asarray(feat, dtype=np.float32))
    W = np.asarray(W, dtype=np.float32)
    label = np.asarray(label)
    Wn = W / np.maximum(
        np.linalg.norm(W, axis=1, keepdims=True), 1e-12
    ).astype(np.float32)
    # wrT[p, q*2+c] = Wn[c, q*128+p]
    wrT = Wn.reshape(C, 4, 128).transpose(2, 1, 0).reshape(128, 8)
    pre = np.zeros((128, 208), dtype=np.float32)
    pre[:, 0:8] = wrT
    pre[:, 8] = S2SIN2
    # bf16 identity packed into f32 cols 137:201
    idb = np.zeros((128, 128), dtype=np.uint16)
    one = _bf16_bits(np.float32(1.0))[()]
    idb[np.arange(128), np.arange(128)] = one
    packed = (idb[:, 0::2].astype(np.uint32)
              | (idb[:, 1::2].astype(np.uint32) << 16)).view(np.float32)
    pre[:, 137:201] = packed
    in_maps = []
    for core in range(NCORES):
        fs = feat[core * BS:(core + 1) * BS]
        ls = label[core * BS:(core + 1) * BS].astype(np.float32)
        ls = ls.reshape(NBLK, 128, SPB).transpose(1, 0, 2).reshape(128, T)
        p = pre.copy()
        p[:, 9:137] = ls
        in_maps.append({"feat": np.ascontiguousarray(fs),
                        "pre": np.ascontiguousarray(p)})
    return in_maps


def _assemble(results):
    outs = []
    for core in range(NCORES):
        o = np.asarray(results[core]["out"])        # [128, T*C] t-major
        o = o.reshape(128, NBLK, SPB, C)            # [p, blk, j, c]
        o = o.transpose(1, 0, 2, 3).reshape(BS, C)  # [blk, p, j, c]
        outs.append(o)
    return np.concatenate(outs, axis=0)


def run(feat, W, label, trace=False, **kw):
    nc = _get_nc()
    in_maps = _make_in_maps(feat, W, label)
    res = run_bass_kernel_spmd(
        nc, in_maps, core_ids=list(range(NCORES)), trace=trace, **kw
    )
    return _assemble(res.results), res


def kernel(feat, W, label):
    out, _ = run(feat, W, label, trace=False)
    return out
